# revision 9
# baseline (speedup 1.0000x reference)
"""Self-contained Trainium2 Bass kernel for the AttentionBlock problem.

Problem (per batch image b of B=4):
    h   = GroupNorm(x[b])                      # [C=512, N=4096], 8 groups
    qkv = W_qkv @ h + b_qkv                    # [1536, N]
    S   = (q + b_q)^T (k)  * C^-0.5            # [N, N]   (k-bias cancels in softmax)
    P   = softmax(S, axis=-1)
    Y   = P @ v^T                              # v-bias folded into proj bias
    out = x[b] + W_proj @ Y^T + b_eff          # b_eff = b_proj + W_proj @ b_v

Sharding: pure data-parallel, no collectives. Core c handles batch b=c//2 and
query-column half h=c%2. Since everything before the output reshape is
permutation-equivariant over the N (spatial) axis, each core receives x[b]
with columns rolled so its half sits at columns [0, 2048); it computes the
full GroupNorm/K/V (needed by every query) and attention outputs only for
columns [0, 2048).

On-chip layout (per core): channels on partitions (4 tiles of 128).
S is computed transposed (S^T[m, n], m=key pos on partitions) so that softmax
denominators come from a ones-vector matmul and P @ V needs no transposes:
v is produced directly in transposed layout vT[m, c] by swapping the matmul
operands. All matmuls run in bf16 with fp32 PSUM accumulation; GroupNorm
statistics are accumulated in fp32.
"""

import numpy as np
import ml_dtypes

import concourse.bacc as bacc
import concourse.tile as tile
import concourse.mybir as mybir
from concourse import bass_utils
from concourse.mybir import AluOpType as OP
from concourse.mybir import ActivationFunctionType as AFT

F32 = mybir.dt.float32
F32R = mybir.dt.float32r
BF16 = mybir.dt.bfloat16
AX = mybir.AxisListType

B, C, H, W = 4, 512, 64, 64
N = H * W              # 4096 spatial positions
NH = N // 2            # per-core query half
CT = C // 128          # 4 channel tiles
MT = N // 128          # 32 key-position tiles
NB = NH // 512         # 4 query blocks of 512
KC = N // 512          # 8 column chunks for full-N passes
GN_N = 64 * N          # elements per group (64 channels x 4096)
SCALE = float(C) ** -0.5
EPS = 1e-5

_CACHE = {}


def build():
    if "nc" in _CACHE:
        return _CACHE["nc"]
    nc = bacc.Bacc("TRN2", target_bir_lowering=False, debug=False, num_devices=8)

    x_d = nc.dram_tensor("x", [C, N], F32, kind="ExternalInput").ap()
    wqkvT_d = nc.dram_tensor("wqkvT", [C, 3 * C], BF16, kind="ExternalInput").ap()
    wprojT_d = nc.dram_tensor("wprojT", [C, C], BF16, kind="ExternalInput").ap()
    gamma_d = nc.dram_tensor("gamma", [128, CT], F32, kind="ExternalInput").ap()
    beta_d = nc.dram_tensor("beta", [128, CT], F32, kind="ExternalInput").ap()
    bq_d = nc.dram_tensor("bq", [128, CT], F32, kind="ExternalInput").ap()
    beff_d = nc.dram_tensor("beff", [128, CT], F32, kind="ExternalInput").ap()
    indA_d = nc.dram_tensor("indA", [128, 2], F32, kind="ExternalInput").ap()
    indB_d = nc.dram_tensor("indB", [2, 128], F32, kind="ExternalInput").ap()
    out_d = nc.dram_tensor("out", [C, NH], F32, kind="ExternalOutput").ap()

    with tile.TileContext(nc) as tc:
        with (
            tc.tile_pool(name="const", bufs=1) as cp,
            tc.tile_pool(name="persist", bufs=1) as pp,
        ):
            # ---- constants / weights
            wq = []
            for ci in range(CT):
                t = cp.tile([128, 3 * C], BF16, name=f"wq{ci}")
                nc.sync.dma_start(t[:], wqkvT_d[ci * 128:(ci + 1) * 128, :])
                wq.append(t)
            wp = []
            for ci in range(CT):
                t = cp.tile([128, C], BF16, name=f"wp{ci}")
                nc.sync.dma_start(t[:], wprojT_d[ci * 128:(ci + 1) * 128, :])
                wp.append(t)
            gamma_t = cp.tile([128, CT], F32, name="gamma_t")
            nc.sync.dma_start(gamma_t[:], gamma_d[:])
            beta_t = cp.tile([128, CT], F32, name="beta_t")
            nc.sync.dma_start(beta_t[:], beta_d[:])
            bq_t = cp.tile([128, CT], F32, name="bq_t")
            nc.sync.dma_start(bq_t[:], bq_d[:])
            beff_t = cp.tile([128, CT], F32, name="beff_t")
            nc.sync.dma_start(beff_t[:], beff_d[:])

            # group-reduction indicator matrices (each 128-partition tile holds
            # two 64-channel groups: p<64 -> group 2*ci, p>=64 -> group 2*ci+1)
            indA = cp.tile([128, 2], F32, name="indA")
            nc.sync.dma_start(indA[:], indA_d[:])
            indB = cp.tile([2, 128], F32, name="indB")
            nc.sync.dma_start(indB[:], indB_d[:])
            ones_bf = cp.tile([128, 1], BF16, name="ones_bf")
            nc.vector.memset(ones_bf[:], 1.0)
            ones1 = cp.tile([1, 128], F32, name="ones1")
            nc.vector.memset(ones1[:], 1.0)
            warm_t = cp.tile([1, 128], BF16, name="warm_t")
            nc.vector.memset(warm_t[:], 1.0)

            # ---- persistent activation storage
            k_bf = [pp.tile([128, N], BF16, name=f"k_bf{ci}") for ci in range(CT)]
            q_bf = [pp.tile([128, NH], BF16, name=f"q_bf{ci}") for ci in range(CT)]
            vT_bf = pp.tile([128, MT * C], BF16, name="vT_bf")  # m-tile m at cols [m*512, (m+1)*512)

            # =========================== phase A ===========================
            with (
                tc.tile_pool(name="gnp", bufs=1) as gp,
                tc.tile_pool(name="psA", bufs=1, space="PSUM") as psA,
            ):
                # PE warm-up (HAM): harmless tiny matmuls while DMA/stats run
                warm_ps = psA.tile([1, 128], F32, name="warm_ps", tag="warm", bufs=1)
                for i in range(100):
                    nc.tensor.matmul(warm_ps[:], warm_t[0:1, 0:1], warm_t[:], start=True, stop=True)

                # x -> bf16 copy (chunked), fp32 stats on the way
                xbf = [gp.tile([128, N], BF16, name=f"xbf{ci}") for ci in range(CT)]
                stats = gp.tile([128, 2 * CT], F32, name="stats")
                sqacc = gp.tile([128, KC * CT], F32, name="sqacc")
                sqs = gp.tile([128, 512], F32, name="sqs", tag="sqs", bufs=2)
                for ci in range(CT):
                    for j in range(KC):
                        xc = gp.tile([128, 512], F32, name="xc", tag="xc", bufs=4)
                        nc.sync.dma_start(xc[:], x_d[ci * 128:(ci + 1) * 128, j * 512:(j + 1) * 512])
                        nc.vector.tensor_copy(xbf[ci][:, j * 512:(j + 1) * 512], xc[:])
                        nc.scalar.activation(
                            sqs[:], xc[:], AFT.Square,
                            accum_out=sqacc[:, ci * KC + j:ci * KC + j + 1],
                        )
                    nc.vector.reduce_sum(stats[:, 2 * ci:2 * ci + 1], xbf[ci][:], AX.X)
                    nc.vector.reduce_sum(
                        stats[:, 2 * ci + 1:2 * ci + 2],
                        sqacc[:, ci * KC:(ci + 1) * KC], AX.X,
                    )

                # cross-partition group reduction via PE
                grp_ps = psA.tile([2, 2 * CT], F32, name="grp_ps", tag="tiny", bufs=2)
                nc.tensor.matmul(grp_ps[:], indA[:], stats[:], start=True, stop=True)

                meanrs = gp.tile([2, 2 * CT], F32, name="meanrs")
                tmp8 = gp.tile([2, 2 * CT], F32, name="tmp8")
                inv_n = 1.0 / float(GN_N)
                nc.vector.tensor_scalar_mul(meanrs[:, 0:CT], grp_ps[:, 0:2 * CT:2], inv_n)
                nc.vector.tensor_scalar_mul(tmp8[:, 0:CT], grp_ps[:, 1:2 * CT:2], inv_n)
                nc.vector.tensor_tensor(tmp8[:, CT:2 * CT], meanrs[:, 0:CT], meanrs[:, 0:CT], OP.mult)
                nc.vector.tensor_tensor(tmp8[:, 0:CT], tmp8[:, 0:CT], tmp8[:, CT:2 * CT], OP.subtract)
                nc.vector.tensor_scalar_add(tmp8[:, 0:CT], tmp8[:, 0:CT], EPS)
                nc.scalar.activation(tmp8[:, CT:2 * CT], tmp8[:, 0:CT], AFT.Sqrt)
                nc.vector.reciprocal(meanrs[:, CT:2 * CT], tmp8[:, CT:2 * CT])

                # broadcast per-group (mean, rstd) back to all 128 partitions
                bc_ps = psA.tile([128, 2 * CT], F32, name="bc_ps", tag="tiny", bufs=2)
                nc.tensor.matmul(bc_ps[:], indB[:], meanrs[:], start=True, stop=True)

                sfac = gp.tile([128, CT], F32, name="sfac")
                tb = gp.tile([128, CT], F32, name="tb")
                nc.vector.tensor_tensor(sfac[:], bc_ps[:, CT:2 * CT], gamma_t[:], OP.mult)
                nc.vector.tensor_tensor(tb[:], bc_ps[:, 0:CT], sfac[:], OP.mult)
                nc.vector.tensor_tensor(tb[:], beta_t[:], tb[:], OP.subtract)

                # Fold the normalization h = x*sfac + tb into the qkv weights:
                #   W' = W * diag(sfac)   (per-contraction-channel scale)
                #   q const: SCALE*(Wq tb + b_q); k const cancels in softmax;
                #   v const (Wv tb + b_v) folds through P@V (rows sum to 1)
                #   into the proj bias: beff2 = beff + W_proj (Wv tb).
                tb_bf = gp.tile([128, CT], BF16, name="tb_bf")
                nc.vector.tensor_copy(tb_bf[:], tb[:])
                # u = Wv tb  (uses unscaled weights; emitted before the scale)
                u_bf = gp.tile([128, CT], BF16, name="u_bf")
                qb2 = gp.tile([128, CT], F32, name="qb2")
                for o in range(CT):
                    uv = psA.tile([128, 1], F32, name="uv", tag="tiny", bufs=2)
                    for ci in range(CT):
                        nc.tensor.matmul(
                            uv[:], wq[ci][:, 2 * C + o * 128:2 * C + (o + 1) * 128],
                            tb_bf[:, ci:ci + 1],
                            start=(ci == 0), stop=(ci == CT - 1),
                        )
                    nc.scalar.copy(u_bf[:, o:o + 1], uv[:])
                    wqt = psA.tile([128, 1], F32, name="wqt", tag="tiny", bufs=2)
                    for ci in range(CT):
                        nc.tensor.matmul(
                            wqt[:], wq[ci][:, o * 128:(o + 1) * 128],
                            tb_bf[:, ci:ci + 1],
                            start=(ci == 0), stop=(ci == CT - 1),
                        )
                    nc.vector.scalar_tensor_tensor(
                        qb2[:, o:o + 1], wqt[:], SCALE, bq_t[:, o:o + 1],
                        OP.mult, OP.add,
                    )
                beff2 = pp.tile([128, CT], F32, name="beff2")  # persists into phase B
                for o in range(CT):
                    bx = psA.tile([128, 1], F32, name="bx", tag="tiny", bufs=2)
                    for ct in range(CT):
                        nc.tensor.matmul(
                            bx[:], wp[ct][:, o * 128:(o + 1) * 128],
                            u_bf[:, ct:ct + 1],
                            start=(ct == 0), stop=(ct == CT - 1),
                        )
                    nc.vector.tensor_tensor(beff2[:, o:o + 1], bx[:], beff_t[:, o:o + 1], OP.add)

                # scale the qkv weights in place (after the matvecs above)
                for ci in range(CT):
                    nc.vector.tensor_scalar_mul(wq[ci][:], wq[ci][:], sfac[:, ci:ci + 1])
                h = xbf

                # ---- qkv projections
                # k[o, n] (no bias: cancels in softmax)
                for nb in range(KC):
                    for ko in range(CT):
                        ps = psA.tile([128, 512], F32, name="ps_k", tag="acc", bufs=4)
                        for ci in range(CT):
                            nc.tensor.matmul(
                                ps[:],
                                wq[ci][:, C + ko * 128:C + (ko + 1) * 128],
                                h[ci][:, nb * 512:(nb + 1) * 512],
                                start=(ci == 0), stop=(ci == CT - 1),
                            )
                        nc.scalar.copy(k_bf[ko][:, nb * 512:(nb + 1) * 512], ps[:])
                # q[o, n] for our half, scaled: q = SCALE*(Wq' h) + qb2
                for nb in range(NB):
                    for qo in range(CT):
                        ps = psA.tile([128, 512], F32, name="ps_q", tag="acc", bufs=4)
                        for ci in range(CT):
                            nc.tensor.matmul(
                                ps[:],
                                wq[ci][:, qo * 128:(qo + 1) * 128],
                                h[ci][:, nb * 512:(nb + 1) * 512],
                                start=(ci == 0), stop=(ci == CT - 1),
                            )
                        nc.scalar.activation(
                            q_bf[qo][:, nb * 512:(nb + 1) * 512], ps[:],
                            AFT.Identity, bias=qb2[:, qo:qo + 1], scale=SCALE,
                        )
                # vT[m, c] directly transposed (h as stationary, Wv'^T moving)
                for m in range(MT):
                    ps = psA.tile([128, 512], F32, name="ps_v", tag="acc", bufs=4)
                    for ci in range(CT):
                        nc.tensor.matmul(
                            ps[:],
                            h[ci][:, m * 128:(m + 1) * 128],
                            wq[ci][:, 2 * C:3 * C],
                            start=(ci == 0), stop=(ci == CT - 1),
                        )
                    nc.scalar.copy(vT_bf[:, m * 512:(m + 1) * 512], ps[:])

            # =========================== phase B ===========================
            with (
                tc.tile_pool(name="att", bufs=1) as ap,
                tc.tile_pool(name="psS", bufs=3, space="PSUM") as psS,
                tc.tile_pool(name="psY", bufs=4, space="PSUM") as psY,
                tc.tile_pool(name="psD", bufs=1, space="PSUM") as psD,
            ):
                xres = []
                for ci in range(CT):
                    t = ap.tile([128, NH], F32, name=f"xres{ci}")
                    nc.sync.dma_start(t[:], x_d[ci * 128:(ci + 1) * 128, 0:NH])
                    xres.append(t)

                for nb in range(NB):
                    yps = [
                        psY.tile([128, 512], F32, name=f"yps{ct}", tag="y", bufs=4)
                        for ct in range(CT)
                    ]
                    dps = psD.tile([1, 512], F32, name="dps", tag="d", bufs=1)
                    for m in range(MT):
                        sps = psS.tile([128, 512], F32, name="sps", tag="s", bufs=3)
                        for ci in range(CT):
                            nc.tensor.matmul(
                                sps[:],
                                k_bf[ci][:, m * 128:(m + 1) * 128],
                                q_bf[ci][:, nb * 512:(nb + 1) * 512],
                                start=(ci == 0), stop=(ci == CT - 1),
                            )
                        es = ap.tile([128, 512], BF16, name="es", tag="es", bufs=4)
                        nc.scalar.activation(es[:], sps[:], AFT.Exp)
                        for ct in range(CT):
                            nc.tensor.matmul(
                                yps[ct][:],
                                vT_bf[:, m * C + ct * 128:m * C + (ct + 1) * 128],
                                es[:],
                                start=(m == 0), stop=(m == MT - 1),
                            )
                        nc.tensor.matmul(
                            dps[:], ones_bf[:], es[:],
                            start=(m == 0), stop=(m == MT - 1),
                        )

                    # normalize: yn[c, n] = Y[c, n] / denom[n]
                    recip = ap.tile([1, 512], F32, name="recip", tag="recip", bufs=2)
                    nc.vector.reciprocal(recip[:], dps[:])
                    rbc_ps = psS.tile([128, 512], F32, name="rbc_ps", tag="s", bufs=3)
                    nc.tensor.matmul(rbc_ps[:], ones1[:], recip[:], start=True, stop=True)
                    rbc = ap.tile([128, 512], F32, name="rbc", tag="rbc", bufs=2)
                    nc.scalar.copy(rbc[:], rbc_ps[:])
                    yn = [
                        ap.tile([128, 512], BF16, name=f"yn{ct}", tag="yn", bufs=8)
                        for ct in range(CT)
                    ]
                    for ct in range(CT):
                        nc.vector.tensor_tensor(yn[ct][:], yps[ct][:], rbc[:], OP.mult)

                    # proj + bias + residual
                    for po in range(CT):
                        pps = psS.tile([128, 512], F32, name="pps", tag="s", bufs=3)
                        for ct in range(CT):
                            nc.tensor.matmul(
                                pps[:],
                                wp[ct][:, po * 128:(po + 1) * 128],
                                yn[ct][:],
                                start=(ct == 0), stop=(ct == CT - 1),
                            )
                        ot = ap.tile([128, 512], F32, name="ot", tag="ot", bufs=4)
                        nc.vector.scalar_tensor_tensor(
                            ot[:], pps[:], beff2[:, po:po + 1],
                            xres[po][:, nb * 512:(nb + 1) * 512],
                            OP.add, OP.add,
                        )
                        nc.sync.dma_start(
                            out_d[po * 128:(po + 1) * 128, nb * 512:(nb + 1) * 512], ot[:]
                        )

    nc.compile()
    _CACHE["nc"] = nc
    return nc


def make_in_maps(x, gamma, beta, w_qkv, b_qkv, w_proj, b_proj):
    x = np.asarray(x, dtype=np.float32).reshape(B, C, N)
    gamma = np.asarray(gamma, dtype=np.float32)
    beta = np.asarray(beta, dtype=np.float32)
    w_qkv = np.asarray(w_qkv, dtype=np.float32)
    b_qkv = np.asarray(b_qkv, dtype=np.float32)
    w_proj = np.asarray(w_proj, dtype=np.float32)
    b_proj = np.asarray(b_proj, dtype=np.float32)

    wqkvT = np.ascontiguousarray(w_qkv.T).astype(ml_dtypes.bfloat16)
    wprojT = np.ascontiguousarray(w_proj.T).astype(ml_dtypes.bfloat16)

    def vec_pt(v):  # [512] -> [128, 4] with [p, t] = v[t*128 + p]
        return np.ascontiguousarray(v.reshape(CT, 128).T, dtype=np.float32)

    gamma2 = vec_pt(gamma)
    beta2 = vec_pt(beta)
    bq2 = vec_pt(SCALE * b_qkv[0:C])
    beff2 = vec_pt(b_proj + w_proj @ b_qkv[2 * C:3 * C])

    p = np.arange(128)
    indA = np.stack([(p < 64), (p >= 64)], axis=1).astype(np.float32)   # [128, 2]
    indB = np.ascontiguousarray(indA.T)                                 # [2, 128]

    in_maps = []
    for core in range(8):
        b, half = core // 2, core % 2
        xroll = np.roll(x[b], -NH * half, axis=1) if half else x[b]
        in_maps.append({
            "x": np.ascontiguousarray(xroll, dtype=np.float32),
            "wqkvT": wqkvT, "wprojT": wprojT,
            "gamma": gamma2, "beta": beta2, "bq": bq2, "beff": beff2,
            "indA": indA, "indB": indB,
        })
    return in_maps


def run(in_maps, trace=False, **kwargs):
    nc = build()
    return bass_utils.run_bass_kernel_spmd(
        nc, in_maps, core_ids=list(range(8)), trace=trace, **kwargs
    )


def kernel(x, gamma, beta, w_qkv, b_qkv, w_proj, b_proj):
    in_maps = make_in_maps(x, gamma, beta, w_qkv, b_qkv, w_proj, b_proj)
    res = run(in_maps)
    out = np.empty((B, C, N), dtype=np.float32)
    for core in range(8):
        b, half = core // 2, core % 2
        out[b][:, half * NH:(half + 1) * NH] = res.results[core]["out"]
    return out.reshape(B, C, H, W)
